# revision 3
# baseline (speedup 1.0000x reference)
"""
Trainium2 kernel for nn_CanonicalLinear (dense_mlp) — v9, split-K.

Reference computation:
    heads[b, n, c] = x @ W[n].T + b[n]          (8 per-head linears)
    out[b, c]      = sum_n heads[b, n, c] * factor[n]

By linearity this collapses to a single linear layer:
    W_eff[c, d] = sum_n factor[n] * W[n, c, d]      (factor folded into W
    b_eff[c]    = sum_n factor[n] * b[n, c]          on the host, so the
    out         = x @ W_eff.T + b_eff                device reduce is adds)

Sharding: DP=2 over batch x TP=4 over C, collective-free: each core
reads its W c-quarter (16.8MB bf16) and batch half of x (16.8MB bf16),
writes its out quarter in bf16 (host upcasts).  ~38MB HBM vs ~110us of
bf16 matmul per core -> compute-bound ridge.

Split-K structure: the contraction (D=2048, 16 k-chunks) is split into
two halves.  Every 128-row batch tile does two passes: wave A
accumulates k0-7 in a PSUM bank and spills the partial (+bias) to a
resident SBUF buffer in bf16; wave B accumulates k8-15 and adds the
partial back on eviction.  This halves the PSUM residency per tile, so
the 8 banks cycle through 64 half-accumulations and the PE runs dense
from ~1/4 into the W stream with no phase cliff:
  - groups 0-3 of the W stream (k0-7) are k-staged into the first 8
    tiles' banks exactly like a classic prologue;
  - the remaining 24 tiles' wave-A passes interleave with the k8-15 W
    stream (their evictions are interleaved with the half-B fold ops on
    DVE so banks recycle promptly);
  - wave B then runs back-to-back over all 32 tiles.
x streams exactly once (the k-halves are disjoint); no on-device
transposes (host supplies x as [P, DK, BS] and W as [N, P, DK, CS]).
"""

import numpy as np

P = 128
B, D, C, N = 8192, 2048, 2048, 8
DP, TP = 2, 4                      # data-parallel x tensor-parallel grid
BS, CS = B // DP, C // TP          # per-core batch rows (4096) / out cols (512)
DK = D // P                        # contraction chunks (16)
KA = 7                             # wave-A contraction chunks
KB = DK - KA                       # wave-B contraction chunks (9)
NBT = BS // P                      # batch tiles per core (32)
NPH1 = 8                           # k-staged prologue tiles
NCORES = DP * TP

_cached_nc = None


def _build(repeat=1):
    import concourse.bass as bass
    import concourse.mybir as mybir
    import concourse.tile as tile
    from concourse import bacc

    FP32 = mybir.dt.float32
    BF16 = mybir.dt.bfloat16
    ADD = mybir.AluOpType.add

    nc = bacc.Bacc()
    xd = nc.dram_tensor("x", [P, DK, BS], BF16, kind="ExternalInput")
    wd = nc.dram_tensor("w", [N, P, DK, CS], BF16, kind="ExternalInput")
    bd = nc.dram_tensor("b", [N, CS], FP32, kind="ExternalInput")
    fd = nc.dram_tensor("f", [N], FP32, kind="ExternalInput")
    od = nc.dram_tensor("out", [BS, CS], BF16, kind="ExternalOutput")

    with tile.TileContext(nc) as tc:
        with (
            tc.tile_pool(name="singles", bufs=1) as singles,
            tc.tile_pool(name="wload", bufs=4) as wload,
            tc.tile_pool(name="waccp", bufs=2) as waccp,
            tc.tile_pool(name="xload", bufs=3) as xload,
            tc.tile_pool(name="outp", bufs=3) as outp,
            tc.tile_pool(name="ps", bufs=8, space="PSUM") as ps,
        ):
            # --- constants ---------------------------------------------
            f_ap = fd[:]
            touch = singles.tile([P, 16], FP32)
            touchg = singles.tile([P, 16], FP32)

            # b_eff[c] = sum_n f[n]*b[n, c] on the PE (K=8 matmul), then
            # broadcast to all partitions (K=1 matmul with a ones row).
            b_sb = singles.tile([N, CS], FP32)
            nc.gpsimd.dma_start(b_sb, bd[:])
            f8 = singles.tile([N, 1], FP32)
            nc.gpsimd.dma_start(
                f8,
                bass.AP(tensor=f_ap.tensor, offset=f_ap.offset,
                        ap=list(f_ap.ap) + [[1, 1]]),
            )
            ones1 = singles.tile([1, P], FP32)
            nc.vector.memset(ones1, 1.0)
            beff_row = singles.tile([1, CS], FP32)
            pw = ps.tile([1, CS], FP32, tag="po")
            nc.tensor.matmul(pw, f8, b_sb)
            nc.any.tensor_copy(beff_row, pw)
            beff = singles.tile([P, CS], FP32)
            pw2 = ps.tile([P, CS], FP32, tag="po")
            nc.tensor.matmul(pw2, ones1, beff_row)
            nc.any.tensor_copy(beff, pw2)

            wd_ap = wd[:]
            HSTR = P * DK * CS          # head stride in wd elements

            for _rep in range(repeat):
                weffT = singles.tile([P, DK, CS], BF16)
                # wave-A partials (bias already folded in), bf16
                part_sb = singles.tile([P, NBT, CS], BF16)
                # x k0..KA-1 for the staged prologue tiles 0-7
                xphA = singles.tile([P, KA, NPH1 * P], BF16)

                acc8 = []
                for i in range(NPH1):
                    acc_i = ps.tile([P, CS], FP32, tag="po", name=f"acc{i}")
                    acc8.append(acc_i)

                # x block loaders (4 tiles per block), split across rings
                xbsA, xbsB = {}, {}

                def load_xbA(j):
                    # wave-A x: tiles 8+4j..11+4j, k-chunks 0..KA-1
                    t0 = NPH1 + 4 * j
                    xb = xload.tile([P, KA, 4 * P], BF16, name=f"xba{j}",
                                    tag="xba", bufs=3)
                    nc.sync.dma_start(
                        xb[:, 0:KA // 2, :],
                        xd[:, 0:KA // 2, t0 * P:(t0 + 4) * P])
                    nc.scalar.dma_start(
                        xb[:, KA // 2:KA, :],
                        xd[:, KA // 2:KA, t0 * P:(t0 + 4) * P])
                    xbsA[j] = xb

                def load_xbB(j):
                    # wave-B x: tiles 4j..4j+3, k-chunks KA..15
                    t0 = 4 * j
                    xb = xload.tile([P, KB, 4 * P], BF16, name=f"xbb{j}",
                                    tag="xbb", bufs=3)
                    nc.sync.dma_start(
                        xb[:, 0:KB // 2, :],
                        xd[:, KA:KA + KB // 2, t0 * P:(t0 + 4) * P])
                    nc.scalar.dma_start(
                        xb[:, KB // 2:KB, :],
                        xd[:, KA + KB // 2:DK, t0 * P:(t0 + 4) * P])
                    xbsB[j] = xb

                def waveA_tile(i):
                    # full k0..KA-1 pass for tile i (>= NPH1) + spill
                    j = (i - NPH1) // 4
                    u = (i - NPH1) % 4
                    xb = xbsA[j]
                    po = ps.tile([P, CS], FP32, tag="po", name=f"poa{i}")
                    for k in range(KA):
                        nc.tensor.matmul(
                            po, xb[:, k, u * P:(u + 1) * P], weffT[:, k, :],
                            start=(k == 0), stop=(k == KA - 1),
                        )
                    nc.vector.tensor_add(part_sb[:, i, :], po, beff)

                # --- W stream: 8 k-pair groups; halves A (g<4) and B ----
                nc.sync.dma_start(xphA[:, 0:1, :], xd[:, 0:1, 0:NPH1 * P])
                nc.scalar.dma_start(xphA[:, 1:2, :], xd[:, 1:2, 0:NPH1 * P])
                for g in range(8):
                    k0, k1 = 2 * g, 2 * g + 2
                    wsp = wload.tile([P, 4, 2, CS], BF16, tag="wbig",
                                     bufs=4)
                    wact = wload.tile([P, 4, 2, CS], BF16, tag="wbig",
                                      bufs=4)
                    for tile_, par, eng in ((wsp, 0, nc.sync),
                                            (wact, 1, nc.scalar)):
                        eng.dma_start(
                            tile_,
                            bass.AP(tensor=wd_ap.tensor,
                                    offset=wd_ap.offset + par * HSTR
                                    + k0 * CS,
                                    ap=[[DK * CS, P], [2 * HSTR, 4],
                                        [CS, 2], [1, CS]]),
                        )
                    nc.gpsimd.tensor_copy(touch[:, g:g + 1],
                                          wsp[:, 0, 0, 0:1])
                    nc.vector.tensor_copy(touchg[:, g:g + 1],
                                          wact[:, 0, 0, 0:1])
                    # prefetch the next xphA k-piece (wave A only)
                    if k1 < KA:
                        ke = min(k1 + 2, KA)
                        xeng = nc.sync if g % 2 == 0 else nc.scalar
                        xeng.dma_start(xphA[:, k1:ke, :],
                                       xd[:, k1:ke, 0:NPH1 * P])
                    if g == 3:
                        load_xbA(0)
                    # fold the two 4-head stacks (GpSimd: sync stack,
                    # DVE: scalar stack + final)
                    p1 = waccp.tile([P, 2, 2, CS], BF16, tag="lvl1", bufs=2)
                    q1 = waccp.tile([P, 2, 2, CS], BF16, tag="lvl1", bufs=2)
                    p2 = waccp.tile([P, 2, CS], BF16, tag="lvl2", bufs=2)
                    q2 = waccp.tile([P, 2, CS], BF16, tag="lvl2", bufs=2)
                    nc.gpsimd.tensor_tensor(p1, wsp[:, 0:2], wsp[:, 2:4],
                                            ADD)
                    nc.gpsimd.tensor_tensor(p2, p1[:, 0], p1[:, 1], ADD)
                    nc.vector.tensor_tensor(q1, wact[:, 0:2], wact[:, 2:4],
                                            ADD)
                    nc.vector.tensor_tensor(q2, q1[:, 0], q1[:, 1], ADD)
                    nc.vector.tensor_tensor(weffT[:, k0:k1, :], p2, q2, ADD)
                    if g < 4:
                        # staged prologue: tiles 0-7 accumulate this group's
                        # wave-A chunks
                        for k in range(k0, min(k1, KA)):
                            for i in range(NPH1):
                                nc.tensor.matmul(
                                    acc8[i],
                                    xphA[:, k, i * P:(i + 1) * P],
                                    weffT[:, k, :],
                                    start=(k == 0),
                                    stop=(k == KA - 1),
                                )
                    else:
                        # half-B stream: interleave wave-A work for 6
                        # tiles per group so PSUM banks recycle promptly
                        if g == 4:
                            load_xbA(1)
                            for i in range(NPH1):
                                nc.vector.tensor_add(part_sb[:, i, :],
                                                     acc8[i], beff)
                        t0 = NPH1 + (g - 4) * 6
                        for i in range(t0, t0 + 6):
                            j = (i - NPH1) // 4
                            if j + 1 < 6 and (j + 1) not in xbsA:
                                load_xbA(j + 1)
                            waveA_tile(i)

                # --- wave B: k8-15 for all 32 tiles, + partial + store --
                load_xbB(0)
                load_xbB(1)
                o_ap = od[:]
                for j in range(NBT // 4):
                    if j + 2 < NBT // 4:
                        load_xbB(j + 2)
                    xb = xbsB.pop(j)
                    osb = outp.tile([P, 4, CS], BF16, tag="osb", bufs=3)
                    for u in range(4):
                        i = 4 * j + u
                        po = ps.tile([P, CS], FP32, tag="po", name=f"pob{i}")
                        for k in range(KB):
                            nc.tensor.matmul(
                                po, xb[:, k, u * P:(u + 1) * P],
                                weffT[:, KA + k, :],
                                start=(k == 0), stop=(k == KB - 1),
                            )
                        nc.vector.tensor_add(osb[:, u, :], po,
                                             part_sb[:, i, :])
                    if j == NBT // 4 - 1:
                        # final block: small stores on both rings
                        for h, heng in ((0, nc.sync), (2, nc.scalar)):
                            heng.dma_start(
                                bass.AP(tensor=o_ap.tensor,
                                        offset=o_ap.offset
                                        + (4 * j + h) * P * CS,
                                        ap=[[CS, P], [P * CS, 2], [1, CS]]),
                                osb[:, h:h + 2, :],
                            )
                    else:
                        oeng = nc.sync if j % 2 == 0 else nc.scalar
                        oeng.dma_start(
                            bass.AP(tensor=o_ap.tensor,
                                    offset=o_ap.offset + 4 * j * P * CS,
                                    ap=[[CS, P], [P * CS, 4], [1, CS]]),
                            osb,
                        )

    nc.finalize()
    return nc


def _get_nc(repeat=1):
    global _cached_nc
    if _cached_nc is None or getattr(_cached_nc, "_repeat", 1) != repeat:
        _cached_nc = _build(repeat=repeat)
        _cached_nc._repeat = repeat
    return _cached_nc


def _shard_inputs(x, W, b, factor):
    from concourse import mybir
    bf16 = mybir.dt.np(mybir.dt.bfloat16)

    in_maps = []
    xsh = {}
    for p in range(DP):
        xs = x[p * BS:(p + 1) * BS].astype(bf16)            # [BS, D]
        xt = np.ascontiguousarray(xs.T)                     # [D, BS]
        xsh[p] = np.ascontiguousarray(
            xt.reshape(DK, P, BS).transpose(1, 0, 2))       # [P, DK, BS]
    # fold the factor into W on the host (elementwise pre-scale fused
    # with the bf16 cast): the device reduce becomes a pure add tree
    Wf = W * factor.astype(np.float32).reshape(N, 1, 1)
    wsh = {}
    for q in range(TP):
        c0 = q * CS
        ws = Wf[:, c0:c0 + CS, :].astype(bf16)              # [N, CS, D]
        wt = ws.transpose(0, 2, 1).reshape(N, DK, P, CS)    # [N, DK, P, CS]
        wsh[q] = np.ascontiguousarray(wt.transpose(0, 2, 1, 3))
    for r in range(NCORES):
        p, q = divmod(r, TP)
        in_maps.append({
            "x": xsh[p],
            "w": wsh[q],
            "b": np.ascontiguousarray(b[:, q * CS:(q + 1) * CS]),
            "f": np.ascontiguousarray(factor),
        })
    return in_maps


def kernel(x, W, b, factor, _trace=False):
    from concourse.bass_utils import run_bass_kernel_spmd

    x = np.asarray(x, dtype=np.float32)
    W = np.asarray(W, dtype=np.float32)
    b = np.asarray(b, dtype=np.float32)
    factor = np.asarray(factor, dtype=np.float32)

    nc = _get_nc()
    in_maps = _shard_inputs(x, W, b, factor)
    res = run_bass_kernel_spmd(nc, in_maps, list(range(NCORES)),
                               trace=_trace)

    out = np.empty((B, C), dtype=np.float32)
    for r in range(NCORES):
        p, q = divmod(r, TP)
        out[p * BS:(p + 1) * BS, q * CS:(q + 1) * CS] = \
            res.results[r]["out"].astype(np.float32)
    if _trace:
        return out, res
    return out
